# revision 16
# baseline (speedup 1.0000x reference)
"""DeBERTa-style disentangled attention head for Trainium2 (Bass/Tile).

Problem: B=8, S=2048, D_MODEL=1024, D_HEAD=64, K2=2048.
Strategy: data-parallel over batch across 8 NeuronCores; per core a
transposed-attention formulation:
  scoresT[j, i] = c2c + c2p + p2c gathered via skew (diagonal-AP) DMAs,
  unsafe softmax (no max subtraction; scores are O(1)), denominator via a
  ones-column folded into the AV matmul, final small transpose.
Matmuls run in float32r (full PE rate, ~1e-4 rel err).
"""
import numpy as np
import bass_rust
import concourse.bass as bass
import concourse.mybir as mybir
import concourse.tile as tile
from concourse.tile import ScopedClock

B, S, D, DH, K2 = 8, 2048, 1024, 64, 2048
NB = S // 128            # 16 blocks of 128
KC = D // 128            # 8 contraction chunks
W_PAD = 4096             # padded width of QRT_pad / KRT'_pad
W_P2C = 2176             # p2c strip width
W_C2P = 1152             # c2p strip width per i-tile (covers 8 j-blocks)
SCALE = float(1.0 / np.sqrt(3 * DH))
NEG = -1e30

F32 = mybir.dt.float32
F32R = mybir.dt.float32r
I32 = mybir.dt.int32

AFT = mybir.ActivationFunctionType
ALU = mybir.AluOpType

# ---------------------------------------------------------------- patches ---

_nop_counter = [0]


def _drain_and_barrier_split(self, tick_clock, wait_clock):
    nc = self.nc
    drain_inst = nc.sync.drain()
    wait_clock.add_sem_waits(
        drain_inst.ins, ScopedClock({None: tick_clock.global_clock})
    )
    si = drain_inst.ins.sync_info
    waits = list(si.on_wait) if si is not None and si.on_wait else []
    if len(waits) > 1:
        drain_inst.ins.sync_info = bass_rust.SyncInfo(
            on_wait=[waits[0]], on_update=list(si.on_update or [])
        )
        for w in waits[1:]:
            d2 = nc.sync.drain()
            d2.ins.sync_info = bass_rust.SyncInfo(on_wait=[w], on_update=[])
    nc.all_engine_barrier()
    assert self.sems is not None
    popped = nc._tile_sem_poison_stack.pop()
    assert popped is self._sem_poison
    nc.clear_and_free_semaphores(list(self.sems.allocated().values()))
    nc.all_engine_barrier()


def _split_excess_waits(nc):
    MAXW = {"EventSemaphore": 2}
    for f in nc.m.functions:
        new_blocks = []
        changed = False
        for bb in f.blocks:
            insts = list(bb.instructions)
            new_insts = []
            bb_changed = False
            for inst in insts:
                si = inst.sync_info
                waits = list(si.on_wait) if si is not None and si.on_wait else []
                cap = MAXW.get(str(inst.opcode), 1)
                if len(waits) > cap:
                    for w in waits[cap:]:
                        _nop_counter[0] += 1
                        nop = bass_rust.InstNoOp(
                            name=f"I-waitsplit-{_nop_counter[0]}", ins=[], outs=[]
                        )
                        nop.engine = inst.engine
                        nop.sync_info = bass_rust.SyncInfo(on_wait=[w], on_update=[])
                        new_insts.append(nop)
                    inst.sync_info = bass_rust.SyncInfo(
                        on_wait=waits[:cap], on_update=list(si.on_update or [])
                    )
                    bb_changed = True
                new_insts.append(inst)
            if bb_changed:
                nb = bass_rust.BasicBlock(name=bb.name, instructions=new_insts)
                nb.IsExit = bb.IsExit
                nb.IsLoopEntry = bb.IsLoopEntry
                nb.IsPredicated = bb.IsPredicated
                new_blocks.append(nb)
                changed = True
            else:
                new_blocks.append(bb)
        if changed:
            f.blocks = new_blocks


tile.TileContext._drain_and_barrier = _drain_and_barrier_split

# ------------------------------------------------------------- AP helpers ---


def _set_ap(ap, pairs, offset=None):
    v = ap.ap
    v.clear()
    for p in pairs:
        v.append(tuple(int(z) for z in p))
    ap.ap = v
    if offset is not None:
        ap.offset = int(offset)
    return ap


def rev_free(ap, n):
    """Reverse the (single) free dim of a 2D AP of width n."""
    out = ap.copy()
    pairs = list(out.ap)
    assert len(pairs) == 2
    (pstep, pcount), (fstep, fcount) = pairs
    assert fstep == 1 and fcount == n
    return _set_ap(out, [(pstep, pcount), (-1, n)], out.offset + n - 1)


def _make_identity(nc, ident):
    nc.gpsimd.memset(ident, 0.0)
    nc.gpsimd.affine_select(
        out=ident,
        in_=ident,
        compare_op=ALU.not_equal,
        fill=1.0,
        base=0,
        pattern=[[-1, ident.shape[0]]],
        channel_multiplier=1,
    )


# ----------------------------------------------------------------- build ----


def build_nc(split_waits=True):
    nc = bass.Bass()
    x_d = nc.dram_tensor("x", [S, D], F32, kind="ExternalInput")
    px_d = nc.dram_tensor("pos_x", [K2, D], F32, kind="ExternalInput")
    mask_d = nc.dram_tensor("padding_mask", [S], I32, kind="ExternalInput")
    w_d = {}
    for w in ("Wq", "Wk", "Wv", "Wqr", "Wkr"):
        w_d[w] = nc.dram_tensor(w, [D, DH], F32, kind="ExternalInput")
    b_d = {}
    for b in ("bq", "bk", "bv", "bqr", "bkr"):
        b_d[b] = nc.dram_tensor(b, [DH], F32, kind="ExternalInput")
    out_d = nc.dram_tensor("out", [S, DH], F32, kind="ExternalOutput")

    with tile.TileContext(nc) as tc:
        with (
            tc.tile_pool(name="consts", bufs=1) as consts,
            tc.tile_pool(name="big", bufs=1) as big,
        ):
            ident32 = consts.tile([128, 128], F32, name="ident32")
            identr = consts.tile([128, 128], F32R, name="identr")
            _make_identity(nc, ident32[:])
            nc.vector.tensor_copy(identr[:], ident32[:])

            # --- weight stacks (lhsT chunks, f32r), bias stacks ---
            # P1 = [Wq | Wk] -> psum rows 0-63 QT, 64-127 KT
            # P3 = [Wkr | Wqr] -> rows 0-63 KRT, 64-127 QRT
            # P4 = [Wv | Wq] -> rows 0-63 VT, 64-127 QT
            wstack = {}
            for sname, (wa, wb) in {
                "P1": ("Wq", "Wk"),
                "P3": ("Wkr", "Wqr"),
                "P4": ("Wv", "Wq"),
            }.items():
                wtmp = consts.tile([128, KC * 128], F32, name=f"wtmp_{sname}", tag="wtmp")
                for kc in range(KC):
                    nc.sync.dma_start(
                        wtmp[:, kc * 128 : kc * 128 + 64],
                        w_d[wa][kc * 128 : (kc + 1) * 128, :],
                    )
                    nc.sync.dma_start(
                        wtmp[:, kc * 128 + 64 : kc * 128 + 128],
                        w_d[wb][kc * 128 : (kc + 1) * 128, :],
                    )
                ws = consts.tile([128, KC * 128], F32R, name=f"ws_{sname}")
                nc.vector.tensor_copy(ws[:], wtmp[:])
                wstack[sname] = ws

            bstack = {}
            for sname, (ba, bb) in {
                "P1": ("bq", "bk"),
                "P3": ("bkr", "bqr"),
                "P4": ("bv", "bq"),
            }.items():
                bt = consts.tile([128, 1], F32, name=f"bs_{sname}")
                nc.sync.dma_start(bt[0:64, :], b_d[ba][:])
                nc.sync.dma_start(bt[64:128, :], b_d[bb][:])
                bstack[sname] = bt

            # --- padding mask -> per-partition bias columns [128, NB] ---
            mrawi = consts.tile([32, 128], I32, name="mrawi")
            nc.vector.memset(mrawi[:], 0)
            mview = mask_d[:].copy()
            _set_ap(mview, [(128, NB), (1, 128)], 0)
            nc.sync.dma_start(mrawi[0:NB, :], mview)
            mrawf = consts.tile([32, 128], F32, name="mrawf")
            nc.vector.tensor_copy(mrawf[:], mrawi[:])

            # --- persistent operand buffers ---
            # BUF1 [128, 2048]: rows 0-63 KT, rows 64-127 KT (dup)
            # BUF2 [128, 4096]: rows 0-63 QT (cols 0-2047), rows 64-127 QRT_pad
            # BUF3 [128, 2048]: rows 0-63 QT, rows 64-127 QT (dup)
            # BUF4 [128, 4096]: rows 0-63 KRT'_pad, rows 64-127 dup
            buf1 = big.tile([128, S], F32R, name="buf1")
            buf2 = big.tile([128, W_PAD], F32R, name="buf2")
            buf3 = big.tile([128, S], F32R, name="buf3")
            buf4 = big.tile([128, W_PAD], F32R, name="buf4")
            vtil = big.tile([128, NB * (DH + 1)], F32R, name="vtil")
            bigbuf = big.tile([128, NB * W_C2P], F32R, name="bigbuf")
            vtsb = bigbuf[0:64, KC * S : KC * S + S]

            # ---------------- phase A: x -> xT -> QT/KT/VT ----------------
            with (
                tc.tile_pool(name="ldpool", bufs=2) as ldpool,
                tc.tile_pool(name="tpsum", bufs=4, space="PSUM") as tpsum,
                tc.tile_pool(name="ppsum", bufs=2, space="PSUM") as ppsum,
            ):
                xtb = bigbuf[:, 0 : KC * S]

                # mask [32, 128] -> [128, 32] via PE, scale to bias
                maskb = consts.tile([128, NB], F32, name="maskb")
                mps = tpsum.tile([128, 32], F32, name="mps", tag="tps")
                nc.tensor.matmul(
                    mps[:], mrawf[:], ident32[0:32, 0:32], is_transpose=True
                )
                nc.vector.tensor_scalar_mul(maskb[:], mps[:, 0:NB], float(NEG))
                def transpose_into_xtb(src_dram):
                    for ib in range(NB):
                        xt = ldpool.tile([128, D], F32, name="xld", tag="xld")
                        nc.sync.dma_start(
                            xt[:], src_dram[ib * 128 : (ib + 1) * 128, :]
                        )
                        for kc in range(KC):
                            ps = tpsum.tile([128, 128], F32, name="tps", tag="tps")
                            nc.tensor.transpose(
                                ps[:], xt[:, kc * 128 : (kc + 1) * 128], ident32[:]
                            )
                            nc.vector.tensor_copy(
                                xtb[:, kc * S + ib * 128 : kc * S + (ib + 1) * 128],
                                ps[:],
                            )

                def project(sname, evict_fns):
                    # evict_fns: list of (rows_lo, fn(psum_slice_ap, nsl))
                    for nsl in range(S // 512):
                        ps = ppsum.tile([128, 512], F32, name="pps", tag="pps")
                        for kc in range(KC):
                            nc.tensor.matmul(
                                ps[:],
                                wstack[sname][:, kc * 128 : (kc + 1) * 128],
                                xtb[:, kc * S + nsl * 512 : kc * S + (nsl + 1) * 512],
                                start=(kc == 0),
                                stop=(kc == KC - 1),
                            )
                        for fn in evict_fns:
                            fn(ps, nsl)

                transpose_into_xtb(x_d)

                def act_evict(dst, src, bias):
                    nc.scalar.activation(dst, src, AFT.Identity, bias=bias, scale=1.0)

                def ev_p1(ps, nsl):
                    sl = slice(nsl * 512, (nsl + 1) * 512)
                    bias = bstack["P1"]
                    # QT -> BUF2 rows 0-63 and BUF3 rows 0-63
                    act_evict(buf2[0:64, sl], ps[0:64, :], bias[0:64, :])
                    act_evict(buf3[0:64, sl], ps[0:64, :], bias[0:64, :])
                    # KT -> BUF1 rows 64-127
                    act_evict(buf1[64:128, sl], ps[64:128, :], bias[64:128, :])

                def ev_p4(ps, nsl):
                    sl = slice(nsl * 512, (nsl + 1) * 512)
                    bias = bstack["P4"]
                    act_evict(vtsb[:, sl], ps[0:64, :], bias[0:64, :])
                    act_evict(buf3[64:128, sl], ps[64:128, :], bias[64:128, :])

                project("P1", [ev_p1])
                project("P4", [ev_p4])

                # KT dup: BUF1 rows 64-127 -> rows 0-63
                nc.sync.dma_start(buf1[0:64, :], buf1[64:128, :])

                # V tiles with ones column
                for jb in range(NB):
                    vp = tpsum.tile([128, 64], F32, name="vps", tag="tps")
                    nc.tensor.matmul(
                        vp[:],
                        vtsb[:, jb * 128 : (jb + 1) * 128].bitcast(F32),
                        ident32[0:64, 0:64],
                        is_transpose=True,
                    )
                    nc.vector.tensor_copy(
                        vtil[:, jb * (DH + 1) : jb * (DH + 1) + DH], vp[:]
                    )
                ones_col = consts.tile([128, 1], F32, name="ones_col")
                nc.vector.memset(ones_col[:], 1.0)
                vones = _set_ap(
                    vtil[:].copy(),
                    [(NB * (DH + 1), 128), (DH + 1, NB), (1, 1)],
                    DH,
                )
                oview = _set_ap(ones_col[:].copy(), [(1, 128), (0, NB), (0, 1)], 0)
                nc.vector.tensor_copy(vones, oview)

                # ---------------- phase A': pos_x -> QRT_pad / KRT'_pad ----
                transpose_into_xtb(px_d)

                def ev_p3(ps, nsl):
                    bias = bstack["P3"]
                    # KRT rows 0-63, reversed into BUF4 main cols, with bias
                    lo = 1024 + (S - 1 - (nsl * 512 + 511))
                    act_evict(
                        buf4[0:64, lo : lo + 512],
                        rev_free(ps[0:64, :], 512),
                        bias[0:64, :],
                    )
                    # QRT rows 64-127 -> BUF2 main cols
                    act_evict(
                        buf2[64:128, 1024 + nsl * 512 : 1024 + (nsl + 1) * 512],
                        ps[64:128, :],
                        bias[64:128, :],
                    )

                project("P3", [ev_p3])

                # clamp pads: broadcast edge columns
                nc.vector.tensor_copy(
                    buf4[0:64, 0:1024], buf4[0:64, 1024:1025].broadcast_to([64, 1024])
                )
                nc.vector.tensor_copy(
                    buf4[0:64, 3072:4096],
                    buf4[0:64, 3071:3072].broadcast_to([64, 1024]),
                )
                nc.vector.tensor_copy(
                    buf2[64:128, 0:1024],
                    buf2[64:128, 1024:1025].broadcast_to([64, 1024]),
                )
                nc.vector.tensor_copy(
                    buf2[64:128, 3072:4096],
                    buf2[64:128, 3071:3072].broadcast_to([64, 1024]),
                )
                # KRT'_pad dup rows 0-63 -> 64-127
                nc.sync.dma_start(buf4[64:128, :], buf4[0:64, :])

            # ---------------- phase C: attention ----------------
            strips = bigbuf
            p2cs = big.tile([128, W_P2C], F32R, name="p2cs")
            expst = big.tile([128, S], F32R, name="expst")

            with (
                tc.tile_pool(name="opsum", bufs=1, space="PSUM") as opsum,
                tc.tile_pool(name="scpsum", bufs=1, space="PSUM") as scpsum,
                tc.tile_pool(name="wpsum", bufs=2, space="PSUM") as wpsum,
                tc.tile_pool(name="skpool", bufs=2) as skpool,
            ):
                outp = opsum.tile([65, S], F32, name="outp")

                for h in (0, 1):
                    # build c2p strips for this half-sweep of j-blocks
                    for it in range(NB):
                        rg = 0 if it % 2 == 0 else 64
                        pm0 = 1024 * h - 128 * it + 1920
                        for c0, w in ((0, 512), (512, 512), (1024, 128)):
                            sp = wpsum.tile([128, 512], F32, name="sps", tag="sps")
                            nc.tensor.matmul(
                                sp[:, 0:w],
                                buf3[rg : rg + 64, it * 128 : (it + 1) * 128],
                                buf4[rg : rg + 64, pm0 + c0 : pm0 + c0 + w],
                                start=True,
                                stop=True,
                                skip_group_check=True,
                            )
                            nc.vector.tensor_copy(
                                strips[:, it * W_C2P + c0 : it * W_C2P + c0 + w],
                                sp[:, 0:w],
                            )

                    for jj in range(8):
                        jb = h * 8 + jj
                        j0 = jb * 128
                        # p2c_attT strip for this j-block
                        su0 = 1920 - j0
                        for c0, w in (
                            (0, 512),
                            (512, 512),
                            (1024, 512),
                            (1536, 512),
                            (2048, 128),
                        ):
                            pp = wpsum.tile([128, 512], F32, name="pps2", tag="sps")
                            nc.tensor.matmul(
                                pp[:, 0:w],
                                buf1[64:128, j0 : j0 + 128],
                                buf2[64:128, su0 + c0 : su0 + c0 + w],
                                start=True,
                                stop=True,
                                skip_group_check=True,
                            )
                            nc.vector.tensor_copy(p2cs[:, c0 : c0 + w], pp[:, 0:w])

                        for ih in (0, 1):
                            sc = scpsum.tile([128, 1024], F32, name="sc", tag="sc")
                            # c2c
                            for nsl in (0, 1):
                                nc.tensor.matmul(
                                    sc[:, nsl * 512 : (nsl + 1) * 512],
                                    buf1[0:64, j0 : j0 + 128],
                                    buf2[
                                        0:64,
                                        ih * 1024
                                        + nsl * 512 : ih * 1024
                                        + (nsl + 1) * 512,
                                    ],
                                    start=True,
                                    stop=False,
                                    skip_group_check=True,
                                )
                            # c2p skew (8 strips at once)
                            c2pn = skpool.tile([128, 1024], F32, name="c2pn", tag="c2pn")
                            sap = strips[:].bitcast(F32)
                            _set_ap(
                                sap,
                                [(NB * W_C2P - 1, 128), (W_C2P, 8), (1, 128)],
                                8 * ih * W_C2P + 128 * jj + 127,
                            )
                            nc.sync.dma_start(c2pn[:], sap)
                            # transpose-accumulate into scores
                            for t in range(8):
                                nc.tensor.matmul(
                                    sc[:, t * 128 : (t + 1) * 128],
                                    c2pn[:, t * 128 : (t + 1) * 128],
                                    ident32[:],
                                    is_transpose=True,
                                    start=False,
                                    stop=True,
                                    skip_group_check=True,
                                )
                            # p2c skew
                            p2ct = skpool.tile([128, 1024], F32R, name="p2ct", tag="p2ct")
                            pap = p2cs[:].copy()
                            _set_ap(
                                pap,
                                [(W_P2C - 1, 128), (1, 1024)],
                                128 + ih * 1024,
                            )
                            nc.sync.dma_start(p2ct[:], pap)
                            # scores += p2cT ; exp(scale*x + maskbias)
                            nc.vector.scalar_tensor_tensor(
                                out=sc[:],
                                in0=sc[:],
                                scalar=1.0,
                                in1=p2ct[:],
                                op0=ALU.mult,
                                op1=ALU.add,
                            )
                            nc.scalar.activation(
                                expst[:, ih * 1024 : (ih + 1) * 1024],
                                sc[:],
                                AFT.Exp,
                                bias=maskb[:, jb : jb + 1],
                                scale=SCALE,
                            )
                        # AV accumulate
                        for nsl in range(4):
                            nc.tensor.matmul(
                                outp[:, nsl * 512 : (nsl + 1) * 512],
                                vtil[:, jb * (DH + 1) : (jb + 1) * (DH + 1)],
                                expst[:, nsl * 512 : (nsl + 1) * 512],
                                start=(jb == 0),
                                stop=(jb == NB - 1),
                                skip_group_check=True,
                            )

                # ---------------- final transpose + normalize ----------------
                # evict outp [65, S] to SBUF, transpose per block to [128, 65]
                # (col 64 = softmax denominator), then per-partition divide.
                outev = expst[0:65, :]
                nc.vector.tensor_copy(outev, outp[:])
                outsb = p2cs[:, 0 : NB * DH].bitcast(F32)
                for t in range(NB):
                    fp = wpsum.tile([128, 512], F32, name="fps", tag="sps")
                    nc.tensor.matmul(
                        fp[:, 0:65],
                        outev[:, t * 128 : (t + 1) * 128].bitcast(F32),
                        ident32[0:65, 0:65],
                        is_transpose=True,
                        skip_group_check=True,
                    )
                    rcol = skpool.tile([128, 1], F32, name="rcol", tag="rcol")
                    nc.vector.reciprocal(rcol[:], fp[:, 64:65])
                    nc.vector.tensor_scalar_mul(
                        outsb[:, t * DH : (t + 1) * DH], fp[:, 0:64], rcol[:]
                    )
                oap = out_d[:].copy()
                _set_ap(oap, [(DH, 128), (128 * DH, NB), (1, DH)], 0)
                nc.sync.dma_start(oap, outsb)

    if split_waits:
        _split_excess_waits(nc)
    nc.finalize()
    return nc


_NC_CACHE = None


def _get_nc():
    global _NC_CACHE
    if _NC_CACHE is None:
        _NC_CACHE = build_nc()
    return _NC_CACHE


def kernel(**inputs):
    from concourse.bass_utils import run_bass_kernel_spmd

    nc = _get_nc()
    in_maps = []
    for b in range(B):
        m = {
            "x": np.ascontiguousarray(inputs["x"][b]),
            "pos_x": np.ascontiguousarray(inputs["pos_x"][b]),
            "padding_mask": np.ascontiguousarray(inputs["padding_mask"][b]),
        }
        for w in ("Wq", "Wk", "Wv", "Wqr", "Wkr"):
            m[w] = np.ascontiguousarray(inputs[w])
        for bn in ("bq", "bk", "bv", "bqr", "bkr"):
            m[bn] = np.ascontiguousarray(inputs[bn])
        in_maps.append(m)
    res = run_bass_kernel_spmd(nc, in_maps, core_ids=list(range(B)))
    return np.stack([r["out"] for r in res.results])
